# revision 1
# baseline (speedup 1.0000x reference)
"""NetXtVLAD consensus kernel for 8 Trainium2 NeuronCores.

Strategy:
  - Data-parallel over batch: 4 batch elements (1200 tokens) per core.
  - Weight folding on host: sa = l2n(x) @ (Ws@We)^T, attn = l2n(x) @ (Wa@We)^T,
    so the soft-assignment path never needs the expanded xe as an input.
  - fp32r matmuls (full-rate on the PE for free-dim >= 256).
  - BN1 batch stats via ones-vector matmuls + a tiny AllReduce (512 B).
  - first_term as per-(batch, group) matmuls contracting over tokens.
  - vlad re-shard via AllToAll (channels sharded), final BN + channel-sharded
    final matmul, AllReduce of the (32, 1024) partials.

Self-contained: hardcodes all shapes; host-side work is limited to layout
permutation / weight folding / shard packing.
"""

import numpy as np

import concourse.bacc as bacc
import concourse.bass as bass
import concourse.mybir as mybir
import concourse.tile as tile
from concourse.bass_utils import run_bass_kernel_spmd
from concourse.masks import make_identity
from concourse.tile_rust import add_dep_helper

F32 = mybir.dt.float32
F32R = mybir.dt.float32r
AF = mybir.ActivationFunctionType
ALU = mybir.AluOpType
AX = mybir.AxisListType

NCORES = 8
B, L, D = 32, 300, 1024
G, C, DE, GD = 8, 64, 2048, 256
BL = 4                      # batches per core
VALID = [128, 128, 44]      # token tiles per batch
NKT = BL * 3                # 12 token tiles per core
NROWS_BN1 = B * L * G       # 76800
EPS_BN = 1e-5
EPS_L2 = 1e-12
QPC = 16                    # (h, c) channel-groups per core (16 * 128 chans)

_CACHE = {}


def _r(ap):
    return ap.bitcast(F32R)


def build_kernel(has_be: bool, has_bias_cat: bool, n_cores: int = NCORES,
                 reps: int = 1):
    key = (has_be, has_bias_cat, n_cores, reps)
    if key in _CACHE:
        return _CACHE[key]

    nc = bacc.Bacc("TRN2", target_bir_lowering=False, debug=False,
                   num_devices=n_cores)

    xt_d = nc.dram_tensor("xt", [NKT, 128, D], F32, kind="ExternalInput")
    xtt_d = nc.dram_tensor("xtt", [NKT, 128, 8, 128], F32R,
                           kind="ExternalInput")
    wet_d = nc.dram_tensor("wet", [8, 128, DE], F32R, kind="ExternalInput")
    wcat_d = nc.dram_tensor("wcat", [8, 128, 520], F32R, kind="ExternalInput")
    wlt_d = nc.dram_tensor("wlt", [QPC, 128, 1024], F32R, kind="ExternalInput")
    cent_d = nc.dram_tensor("cent", [C, GD], F32, kind="ExternalInput")
    fbnp_d = nc.dram_tensor("fbnp", [32, 128], F32, kind="ExternalInput")
    bn1gb_d = nc.dram_tensor("bn1gb", [1, 128], F32, kind="ExternalInput")
    bl8_d = nc.dram_tensor("bl8", [1, 1024], F32, kind="ExternalInput")
    if has_bias_cat:
        bias_cat_d = nc.dram_tensor("biascat", [1, 520], F32,
                                    kind="ExternalInput")
    if has_be:
        be_d = nc.dram_tensor("bevec", [1, DE], F32, kind="ExternalInput")
    out_d = nc.dram_tensor("out", [32, 1024], F32, kind="ExternalOutput")

    group = [list(range(n_cores))]

    def _collective(kind, op, ins, outs):
        if n_cores == 1:
            nc.scalar.dma_start(out=outs[0], in_=ins[0])
        else:
            nc.gpsimd.collective_compute(kind, op, replica_groups=group,
                                         ins=[ins[0].opt()], outs=[outs[0].opt()])

    with tile.TileContext(nc) as tc:
      for _rep in range(reps):
            with tc.tile_pool(name="const", bufs=1) as cpool, \
                 tc.tile_pool(name="dram", bufs=1, space="DRAM") as dpool, \
                 tc.tile_pool(name="sa", bufs=NKT) as sapool, \
                 tc.tile_pool(name="wet", bufs=1) as wetpool:

                # ---------- P0: constants ----------
                ident = cpool.tile([128, 128], F32)
                make_identity(nc, ident)
                ones_f = cpool.tile([128, 1], F32)
                nc.vector.memset(ones_f, 1.0)
                ones = cpool.tile([128, 1], F32R)
                nc.vector.tensor_copy(out=ones, in_=ones_f)
                ones2 = cpool.tile([128, 2], F32R)
                nc.vector.tensor_copy(out=ones2[:, 0:1], in_=ones_f)
                nc.vector.tensor_copy(out=ones2[:, 1:2], in_=ones_f)
                epsbn = cpool.tile([128, 1], F32)
                nc.vector.memset(epsbn, EPS_BN)
                eps12 = cpool.tile([128, 1], F32)
                nc.vector.memset(eps12, EPS_L2)

                cent_sb = cpool.tile([C, GD], F32)
                nc.sync.dma_start(out=cent_sb, in_=cent_d[:, :])
                bn1gb_sb = cpool.tile([1, 128], F32)
                nc.sync.dma_start(out=bn1gb_sb, in_=bn1gb_d[:, :])
                bl8_row = cpool.tile([1, 1024], F32)
                nc.sync.dma_start(out=bl8_row, in_=bl8_d[:, :])
                bl8_bc = cpool.tile([32, 1024], F32)
                nc.gpsimd.partition_broadcast(bl8_bc, bl8_row)

                fbnp_sb = cpool.tile([32, 128], F32)
                nc.sync.dma_start(out=fbnp_sb, in_=fbnp_d[:, :])

                if has_bias_cat:
                    bc_row = cpool.tile([1, 520], F32)
                    nc.sync.dma_start(out=bc_row, in_=bias_cat_d[:, :])
                    bc_bc = cpool.tile([128, 520], F32)
                    nc.gpsimd.partition_broadcast(bc_bc, bc_row)
                if has_be:
                    be_row = cpool.tile([1, DE], F32)
                    nc.sync.dma_start(out=be_row, in_=be_d[:, :])
                    be_bc = cpool.tile([128, DE], F32)
                    nc.gpsimd.partition_broadcast(be_bc, be_row)

                s_all = cpool.tile([128, NKT], F32)
                sig_all = cpool.tile([128, NKT, 8], F32)
                vladT = cpool.tile([128, 128, BL], F32)  # [d_low, q, b]
                stats_sb = cpool.tile([1, 128], F32)
                gstats_sb = cpool.tile([1, 128], F32)
                scale_bc = cpool.tile([128, 512], F32)
                shift_bc = cpool.tile([128, 512], F32)

                # wet tile allocated up front; DMA emitted after P1 so the
                # x/wcat loads win the DMA queues at startup
                wet_sb = wetpool.tile([128, 8, DE], F32R)

                # DRAM bounce buffers
                stats_in = dpool.tile([1, 128], F32)
                stats_out = dpool.tile([1, 128], F32)
                a2a_in = dpool.tile([NCORES, 128, QPC, BL], F32)
                a2a_out = dpool.tile([NCORES, 128, QPC, BL], F32)
                ar_in = dpool.tile([32, 1024], F32)
                ar_out = dpool.tile([32, 1024], F32)

                # transposed final-BN params [128, 32] (cols 0:16 gamma, 16:32 beta)
                with tc.tile_pool(name="p0ps", bufs=1, space="PSUM") as p0ps:
                    fps = p0ps.tile([128, 32], F32)
                    nc.tensor.transpose(fps, fbnp_sb, ident[:32, :32])
                    fbnT = cpool.tile([128, 32], F32)
                    nc.vector.tensor_copy(out=fbnT, in_=fps)

                sa_tiles = []

                # ---------- P1: sa matmuls + BN1 partial stats ----------
                with tc.tile_pool(name="wcat", bufs=1) as wcatpool, \
                     tc.tile_pool(name="p1t", bufs=3) as p1t, \
                     tc.tile_pool(name="p1xT", bufs=4) as p1xT, \
                     tc.tile_pool(name="p1scr", bufs=2) as p1scr, \
                     tc.tile_pool(name="p1small", bufs=4) as p1small, \
                     tc.tile_pool(name="p1ps", bufs=2, space="PSUM") as p1ps, \
                     tc.tile_pool(name="p1stats", bufs=1, space="PSUM") as p1statsps:

                    wcat_sb = wcatpool.tile([128, 8, 520], F32R)
                    for wk in range(8):
                        nc.sync.dma_start(out=wcat_sb[:, wk, :],
                                          in_=wcat_d[wk, :, :])

                    stats1 = p1statsps.tile([1, 512], F32, tag="st1")
                    stats2 = p1statsps.tile([1, 512], F32, tag="st2")

                    # prefetch transposed-x tiles first: they gate the PE
                    xTs = []
                    for kt in range(NKT):
                        xT = p1xT.tile([128, 8, 128], F32R, tag="xT")
                        xTs.append(xT)
                        nc.sync.dma_start(out=xT, in_=xtt_d[kt, :, :, :])

                    # P1-A: l2-norm factors for all tiles (contiguous run
                    # on the sqrt ACT table-set)
                    for kt in range(NKT):
                        xnat = p1t.tile([128, D], F32, tag="xnat")
                        nc.sync.dma_start(out=xnat, in_=xt_d[kt, :, :])
                        scr = p1scr.tile([128, D], F32, tag="scr")
                        ssq = p1small.tile([128, 1], F32, tag="ssq")
                        nc.scalar.activation(out=scr, in_=xnat, func=AF.Square,
                                             accum_out=ssq)
                        nrm = p1small.tile([128, 1], F32, tag="nrm")
                        nc.scalar.activation(out=nrm, in_=ssq, func=AF.Sqrt)
                        nc.vector.tensor_tensor(out=nrm, in0=nrm, in1=eps12,
                                                op=ALU.max)
                        last_a = nc.vector.reciprocal(
                            out=s_all[:, kt:kt + 1], in_=nrm)

                    # P1-B: soft-assignment matmuls + BN1 stats + attn gates
                    for kt in range(NKT):
                        ci = kt % 3
                        K = VALID[ci]
                        xT = xTs[kt]

                        # sa = xT.T @ wcat  (accumulate over d chunks)
                        saps = p1ps.tile([128, 520], F32, tag="saps")
                        for k in range(8):
                            nc.tensor.matmul(saps[:, 0:512], xT[:, k, :],
                                             wcat_sb[:, k, 0:512],
                                             start=(k == 0), stop=(k == 7))
                            nc.tensor.matmul(saps[:, 512:520], xT[:, k, :],
                                             wcat_sb[:, k, 512:520],
                                             start=(k == 0), stop=(k == 7))

                        sa_t = sapool.tile([128, 520], F32R, tag="sa")
                        sa_tiles.append(sa_t)
                        if has_bias_cat:
                            nc.vector.tensor_scalar_mul(
                                out=sa_t, in0=saps,
                                scalar1=s_all[:, kt:kt + 1])
                        else:
                            nc.scalar.mul(out=sa_t, in_=saps,
                                          mul=s_all[:, kt:kt + 1])
                        if has_bias_cat:
                            nc.vector.tensor_tensor(out=sa_t, in0=sa_t, in1=bc_bc,
                                                    op=ALU.add)

                        if kt == 3:
                            # stream the expansion weights in while the sa
                            # matmuls run; needed from P3 onwards (chunked so
                            # small control DMAs are not stuck behind it)
                            for wk in range(8):
                                nc.sync.dma_start(out=wet_sb[:, wk, :],
                                                  in_=wet_d[wk, :, :])
                        sq = p1scr.tile([128, 512], F32R, tag="sq")
                        nc.vector.tensor_mul(out=sq, in0=sa_t.bitcast(F32)[:, 0:512],
                                             in1=sa_t.bitcast(F32)[:, 0:512])
                        nc.tensor.matmul(stats1, ones[:K], sa_t[:K, 0:512],
                                         start=(kt == 0), stop=(kt == NKT - 1))
                        nc.tensor.matmul(stats2, ones[:K], sq[:K],
                                         start=(kt == 0), stop=(kt == NKT - 1))

                    # chunk-reduce stats to 64 channels: channel j = cols {j+64*ch}
                    nc.vector.tensor_reduce(
                        out=stats_sb[0:1, 0:64],
                        in_=stats1.rearrange("p (ch j) -> p j ch", ch=8),
                        axis=AX.X, op=ALU.add)
                    nc.vector.tensor_reduce(
                        out=stats_sb[0:1, 64:128],
                        in_=stats2.rearrange("p (ch j) -> p j ch", ch=8),
                        axis=AX.X, op=ALU.add)
                    nc.scalar.dma_start(out=stats_in, in_=stats_sb)
                    _collective("AllReduce", ALU.add, [stats_in], [stats_out])
                    nc.scalar.dma_start(out=gstats_sb, in_=stats_out)

                # ---------- BN1 global affine ----------
                inv_n = 1.0 / float(NROWS_BN1)
                mu = cpool.tile([1, 64], F32)
                nc.scalar.mul(out=mu, in_=gstats_sb[0:1, 0:64], mul=inv_n)
                m2 = cpool.tile([1, 64], F32)
                nc.scalar.mul(out=m2, in_=gstats_sb[0:1, 64:128], mul=inv_n)
                var = cpool.tile([1, 64], F32)
                nc.vector.tensor_mul(out=var, in0=mu, in1=mu)
                nc.vector.tensor_sub(out=var, in0=m2, in1=var)
                sd = cpool.tile([1, 64], F32)
                sd_inst = nc.scalar.activation(out=sd, in_=var, func=AF.Sqrt,
                                               bias=epsbn[0:1])
                rstd = cpool.tile([1, 64], F32)
                nc.vector.reciprocal(out=rstd, in_=sd)
                scale_r = cpool.tile([1, 64], F32)
                nc.vector.tensor_mul(out=scale_r, in0=bn1gb_sb[0:1, 0:64],
                                     in1=rstd)
                shift_r = cpool.tile([1, 64], F32)
                nc.vector.tensor_mul(out=shift_r, in0=mu, in1=scale_r)
                nc.vector.tensor_sub(out=shift_r, in0=bn1gb_sb[0:1, 64:128],
                                     in1=shift_r)
                # tile 8x along the free dim, then broadcast to all partitions
                scale_cols = cpool.tile([1, 512], F32)
                nc.vector.tensor_copy(
                    out=scale_cols.rearrange("p (ch j) -> p ch j", ch=8),
                    in_=bass.AP(tensor=scale_r.tensor, offset=scale_r.offset,
                                ap=[scale_r.ap[0], [0, 8], [1, 64]]))
                shift_cols = cpool.tile([1, 512], F32)
                nc.vector.tensor_copy(
                    out=shift_cols.rearrange("p (ch j) -> p ch j", ch=8),
                    in_=bass.AP(tensor=shift_r.tensor, offset=shift_r.offset,
                                ap=[shift_r.ap[0], [0, 8], [1, 64]]))
                nc.gpsimd.partition_broadcast(scale_bc, scale_cols)
                nc.gpsimd.partition_broadcast(shift_bc, shift_cols)

                # attention gates for all tiles: sigmoid = 1/(1+exp(-logit));
                # pinned after the BN1 sqrt so the ACT engine switches
                # table-sets exactly once before the exp runs
                with tc.tile_pool(name="p1c", bufs=4) as p1c:
                    for kt in range(NKT):
                        ea = p1c.tile([128, 8], F32, tag="ea")
                        ea_inst = nc.scalar.activation(
                            out=ea, in_=sa_tiles[kt].bitcast(F32)[:, 512:520],
                            func=AF.Exp, scale=-1.0)
                        add_dep_helper(ea_inst.ins, sd_inst.ins,
                                       reason="batch exps after sqrt-set")
                        nc.vector.tensor_scalar_add(out=ea, in0=ea,
                                                    scalar1=ones_f)
                        nc.vector.reciprocal(out=sig_all[:, kt, :], in_=ea)

                # ---------- P3: xe + softmax + first_term + vlad ----------
                with tc.tile_pool(name="p3t", bufs=2) as p3t, \
                     tc.tile_pool(name="p3xT", bufs=5) as p3xT, \
                     tc.tile_pool(name="p3xe", bufs=5) as p3xe, \
                     tc.tile_pool(name="p3e", bufs=3) as p3e, \
                     tc.tile_pool(name="p3act", bufs=3) as p3act, \
                     tc.tile_pool(name="p3small", bufs=8) as p3small, \
                     tc.tile_pool(name="p3vlad", bufs=6) as p3vlad, \
                     tc.tile_pool(name="p3ps", bufs=2, space="PSUM") as p3ps, \
                     tc.tile_pool(name="p3ft", bufs=2, space="PSUM") as p3ft:

                    vlads = [None] * BL
                    ssq_cols = cpool.tile([C, BL], F32)
                    # software pipeline: xe production runs PD tiles ahead of
                    # the (AllReduce-gated) softmax/first-term consumption, so
                    # the PE keeps streaming matmuls through the sync stall
                    PD = 4
                    xe_tiles = [None] * NKT
                    ftps = asps = None
                    for step in range(NKT + PD):
                      if step < NKT:
                        kt = step
                        xT = p3xT.tile([128, 8, 128], F32R, tag="xT3")
                        nc.sync.dma_start(out=xT, in_=xtt_d[kt, :, :, :])
                        xe = p3xe.tile([128, DE], F32R, tag="xe")
                        xe_tiles[kt] = xe
                        for n in range(4):
                            xps = p3ps.tile([128, 512], F32, tag="xeps")
                            for k in range(8):
                                nc.tensor.matmul(
                                    xps, xT[:, k, :],
                                    wet_sb[:, k, n * 512:(n + 1) * 512],
                                    start=(k == 0), stop=(k == 7))
                            if n % 2 == 0 or has_be:
                                nc.vector.tensor_scalar_mul(
                                    out=xe[:, n * 512:(n + 1) * 512], in0=xps,
                                    scalar1=s_all[:, kt:kt + 1])
                            else:
                                nc.scalar.mul(out=xe[:, n * 512:(n + 1) * 512],
                                              in_=xps,
                                              mul=s_all[:, kt:kt + 1])
                            if has_be:
                                nc.vector.tensor_tensor(
                                    out=xe[:, n * 512:(n + 1) * 512],
                                    in0=xe[:, n * 512:(n + 1) * 512],
                                    in1=be_bc[:, n * 512:(n + 1) * 512],
                                    op=ALU.add)
                      if step >= PD:
                        kt = step - PD
                        b, ci = divmod(kt, 3)
                        K = VALID[ci]
                        if ci == 0:
                            ftps = p3ft.tile([C, GD], F32, tag="ft")
                            asps = p3ft.tile([C, 2], F32, tag="asum")
                        xe = xe_tiles[kt]
                        if True:
                            sa_t = sa_tiles[kt]
                            # z = sa*scale + shift ; e = exp(z)
                            e = p3e.tile([128, 512], F32, tag="e")
                            nc.vector.tensor_tensor(out=e, in0=sa_t.bitcast(F32)[:, 0:512],
                                                    in1=scale_bc, op=ALU.mult)
                            nc.vector.tensor_tensor(out=e, in0=e, in1=shift_bc,
                                                    op=ALU.add)
                            nc.scalar.activation(out=e, in_=e, func=AF.Exp)

                            den = p3small.tile([128, 8], F32, tag="den")
                            nc.vector.tensor_reduce(
                                out=den,
                                in_=e.rearrange("p (c g) -> p g c", g=8),
                                axis=AX.X, op=ALU.add)
                            rden = p3small.tile([128, 8], F32, tag="rden")
                            nc.vector.reciprocal(out=rden, in_=den)
                            w = p3small.tile([128, 8], F32, tag="w")
                            nc.vector.tensor_mul(out=w,
                                                 in0=sig_all[:, kt, :],
                                                 in1=rden)

                            # act = e * w  (broadcast over c), written as f32r
                            act = p3act.tile([128, 512], F32R, tag="act")
                            nc.vector.tensor_tensor(
                                out=act.rearrange("p (c g) -> p c g", g=8),
                                in0=e.rearrange("p (c g) -> p c g", g=8),
                                in1=bass.AP(tensor=w.tensor, offset=w.offset,
                                            ap=[w.ap[0], [0, 64], [1, 8]]),
                                op=ALU.mult)

                            # first_term accumulation
                            e_r3 = act.rearrange("p (c g) -> p g c", g=8)
                            for g in range(G):
                                nc.tensor.matmul(
                                    ftps, e_r3[:K, g, :],
                                    xe[:K, g * 256:(g + 1) * 256],
                                    start=(ci == 0 and g == 0),
                                    stop=(ci == 2 and g == 7))
                            gred = p3small.tile([128, 64], F32R, tag="gred")
                            with nc.allow_low_precision(
                                    reason="8-term reduce rounded to f32r"):
                                nc.vector.tensor_reduce(
                                    out=gred,
                                    in_=act.bitcast(F32).rearrange(
                                        "p (c g) -> p c g", g=8),
                                    axis=AX.X, op=ALU.add)
                            nc.tensor.matmul(asps, gred[:K], ones2[:K],
                                             start=(ci == 0), stop=(ci == 2))

                        if ci == 2:
                            # vlad_b = ft - asum*centroids ; squared norms
                            sterm = p3vlad.tile([C, GD], F32, tag="sterm")
                            nc.vector.tensor_scalar_mul(out=sterm, in0=cent_sb,
                                                        scalar1=asps[:, 0:1])
                            vlad = p3vlad.tile([C, GD], F32, tag="vlad")
                            vlads[b] = vlad
                            nc.vector.tensor_sub(out=vlad, in0=ftps, in1=sterm)
                            vsq = p3vlad.tile([C, GD], F32, tag="vsq")
                            nc.vector.tensor_mul(out=vsq, in0=vlad, in1=vlad)
                            nc.vector.tensor_reduce(out=ssq_cols[:, b:b + 1],
                                                    in_=vsq,
                                                    axis=AX.X, op=ALU.add)

                    # batched l2 normalization of vlad (exact 1/max(sqrt(s),eps))
                    nrm2 = cpool.tile([C, BL], F32)
                    nc.scalar.activation(out=nrm2, in_=ssq_cols, func=AF.Sqrt)
                    nc.vector.tensor_scalar_max(out=nrm2, in0=nrm2,
                                                scalar1=eps12[:C])
                    rn = cpool.tile([C, BL], F32)
                    nc.vector.reciprocal(out=rn, in_=nrm2)
                    for b in range(BL):
                        nc.vector.tensor_scalar_mul(out=vlads[b], in0=vlads[b],
                                                    scalar1=rn[:, b:b + 1])
                        for h in range(2):
                            tp = p3ps.tile([128, 128], F32, tag="tp3")
                            nc.tensor.transpose(
                                tp[:, 0:64], vlads[b][:, h * 128:(h + 1) * 128],
                                ident[:64, :64])
                            nc.vector.tensor_copy(
                                out=vladT[:, h * 64:(h + 1) * 64, b],
                                in_=tp[:, 0:64])

                    nc.sync.dma_start(
                        out=a2a_in[:, :, :, :].rearrange("d p q b -> p d q b"),
                        in_=vladT.rearrange("p (d q) b -> p d q b", d=NCORES))
                    _collective("AllToAll", ALU.bypass, [a2a_in], [a2a_out])

                # ---------- P4: final BN + final matmul ----------
                with tc.tile_pool(name="wlt", bufs=1) as wltpool, \
                     tc.tile_pool(name="p4", bufs=2) as p4pool, \
                     tc.tile_pool(name="p4small", bufs=8) as p4small, \
                     tc.tile_pool(name="p4ps", bufs=2, space="PSUM") as p4ps:

                    wlt_sb = wltpool.tile([128, QPC, 1024], F32R)
                    for q in range(QPC):
                        nc.sync.dma_start(out=wlt_sb[:, q, :],
                                          in_=wlt_d[q, :, :])

                    vchunk = p4pool.tile([128, NCORES, QPC, BL], F32, tag="vchunk")
                    nc.sync.dma_start(
                        out=vchunk,
                        in_=a2a_out[:, :, :, :].rearrange("s p q b -> p s q b"))
                    vbn = p4pool.tile([128, QPC, 32], F32R, tag="vbn")

                    for q in range(QPC):
                        vflat = p4small.tile([128, 32], F32, tag="vflat")
                        nc.vector.tensor_copy(
                            out=vflat.rearrange("p (s b) -> p s b", b=BL),
                            in_=vchunk[:, :, q, :])
                        bnst = p4small.tile([128, 6], F32, tag="bnst")
                        nc.vector.bn_stats(out=bnst, in_=vflat)
                        mv = p4small.tile([128, 2], F32, tag="mv")
                        nc.vector.bn_aggr(out=mv, in_=bnst)
                        sdq = p4small.tile([128, 1], F32, tag="sdq")
                        nc.scalar.activation(out=sdq, in_=mv[:, 1:2], func=AF.Sqrt,
                                             bias=epsbn)
                        rsq = p4small.tile([128, 1], F32, tag="rsq")
                        nc.vector.reciprocal(out=rsq, in_=sdq)
                        scq = p4small.tile([128, 1], F32, tag="scq")
                        nc.vector.tensor_mul(out=scq, in0=fbnT[:, q:q + 1],
                                             in1=rsq)
                        shq = p4small.tile([128, 1], F32, tag="shq")
                        nc.vector.tensor_mul(out=shq, in0=mv[:, 0:1], in1=scq)
                        nc.vector.tensor_sub(out=shq,
                                             in0=fbnT[:, 16 + q:17 + q], in1=shq)
                        nc.vector.tensor_scalar(out=vbn[:, q, :], in0=vflat,
                                                scalar1=scq, scalar2=shq,
                                                op0=ALU.mult, op1=ALU.add)

                    out_sb = p4pool.tile([32, 1024], F32, tag="outsb")
                    for n in range(2):
                        fpsm = p4ps.tile([32, 512], F32, tag="fin")
                        for q in range(QPC):
                            nc.tensor.matmul(
                                fpsm, vbn[:, q, :],
                                wlt_sb[:, q, n * 512:(n + 1) * 512],
                                start=(q == 0), stop=(q == QPC - 1))
                        nc.vector.tensor_tensor(
                            out=out_sb[:, n * 512:(n + 1) * 512], in0=fpsm,
                            in1=bl8_bc[:, n * 512:(n + 1) * 512], op=ALU.add)

                    nc.scalar.dma_start(out=ar_in, in_=out_sb)
                    _collective("AllReduce", ALU.add, [ar_in], [ar_out])
                    nc.scalar.dma_start(out=out_d[:, :], in_=ar_out)

    nc.finalize()
    _CACHE[key] = nc
    return nc


def _prep_inputs(x, We, be, Ws, bn1_g, bn1_b, Wa, ba, centroids,
                 fbn_g, fbn_b, Wl, bl):
    f = np.float32
    x = np.asarray(x, f)
    We = np.asarray(We, f)
    Ws = np.asarray(Ws, f)
    Wa = np.asarray(Wa, f)
    be = np.asarray(be, f)
    ba = np.asarray(ba, f)
    Wl = np.asarray(Wl, f)

    WsWe = Ws @ We                       # (512, 1024)
    WaWe = Wa @ We                       # (8, 1024)
    Wcat = np.concatenate([WsWe, WaWe], 0)          # (520, 1024)
    WcatT = np.ascontiguousarray(Wcat.T).reshape(8, 128, 520)
    WeT = np.ascontiguousarray(We.T).reshape(8, 128, DE)

    bias_cat = np.concatenate([Ws @ be, Wa @ be + ba]).reshape(1, 520)
    has_bias_cat = bool(np.any(bias_cat))
    has_be = bool(np.any(be))

    # permuted channel order: p_idx = (h*64 + c)*128 + d_low,
    # original chan = c*256 + h*128 + d_low
    Wlp = np.ascontiguousarray(
        Wl.reshape(1024, C, 2, 128).transpose(2, 1, 3, 0).reshape(16384, 1024))
    fg = np.ascontiguousarray(
        np.asarray(fbn_g, f).reshape(C, 2, 128).transpose(1, 0, 2).reshape(128, 128))
    fb = np.ascontiguousarray(
        np.asarray(fbn_b, f).reshape(C, 2, 128).transpose(1, 0, 2).reshape(128, 128))

    bn1gb = np.concatenate([np.asarray(bn1_g, f),
                            np.asarray(bn1_b, f)]).reshape(1, 128)
    bl8 = (np.asarray(bl, f) / 8.0).reshape(1, 1024)
    cent = np.ascontiguousarray(np.asarray(centroids, f))

    in_maps = []
    for j in range(NCORES):
        xj = x[j * BL:(j + 1) * BL]          # (4, 300, 1024)
        xt = np.ones((NKT, 128, D), f)
        for b in range(BL):
            for ci in range(3):
                v = VALID[ci]
                xt[b * 3 + ci, :v] = xj[b, ci * 128:ci * 128 + v]
        xtt = np.ascontiguousarray(
            xt.reshape(NKT, 128, 8, 128).transpose(0, 3, 2, 1))
        fbnp = np.concatenate([fg[j * QPC:(j + 1) * QPC],
                               fb[j * QPC:(j + 1) * QPC]], 0)  # (32, 128)
        wlt = np.ascontiguousarray(
            Wlp[j * 2048:(j + 1) * 2048].reshape(QPC, 128, 1024))
        m = {"xt": np.ascontiguousarray(xt), "xtt": xtt,
             "wet": WeT, "wcat": WcatT,
             "wlt": wlt, "cent": cent, "fbnp": np.ascontiguousarray(fbnp),
             "bn1gb": bn1gb, "bl8": bl8}
        if has_bias_cat:
            m["biascat"] = bias_cat
        if has_be:
            m["bevec"] = be.reshape(1, DE)
        in_maps.append(m)
    return in_maps, has_be, has_bias_cat


def kernel(**inputs):
    in_maps, has_be, has_bias_cat = _prep_inputs(**inputs)
    nc = build_kernel(has_be, has_bias_cat)
    res = run_bass_kernel_spmd(nc, in_maps, core_ids=list(range(NCORES)))
    out = np.ascontiguousarray(np.asarray(res.results[0]["out"], np.float32))
    return out



# revision 17
# speedup vs baseline: 1.3898x; 1.3898x over previous
"""NetXtVLAD consensus kernel for 8 Trainium2 NeuronCores.

Strategy (v2 — factored first_term, bf16 datapath):
  - Data-parallel over batch: 4 batch elements (1200 tokens) per core.
  - Weight folding on host: sa/attn logits come from x @ (Ws@We)^T and
    x @ (Wa@We)^T, so the expanded xe is never needed for the gating path.
  - first_term is factored as stage1 y[b,cg,:] = sum_l act[l,cg] x_hat[l,:]
    (contract over tokens) followed by stage2 ft = sum_g y_g @ We_g^T
    (contract over D). This is ~3x fewer PE cycles than materializing xe.
  - All large matmul operands are bf16 (validated ~4e-3 rel err, gate 2e-2);
    accumulation stays f32 in PSUM.
  - BN1 batch stats via 1/N-scaled ones-vector matmuls + a tiny AllReduce.
  - vlad is produced directly in transposed (d-major) layout by stage2, so
    the AllToAll re-shard needs no PE transposes.
  - Final BN + channel-sharded final matmul, AllReduce of (32,1024) partials.

Self-contained: hardcodes all shapes; host-side work is limited to layout
permutation / weight folding / shard packing.
"""

import numpy as np
from ml_dtypes import bfloat16

import concourse.bacc as bacc
import concourse.bass as bass
import concourse.mybir as mybir
import concourse.tile as tile
from concourse.bass_utils import run_bass_kernel_spmd
from concourse.masks import make_identity

F32 = mybir.dt.float32
F32R = mybir.dt.float32r
BF16 = mybir.dt.bfloat16
AF = mybir.ActivationFunctionType
ALU = mybir.AluOpType
AX = mybir.AxisListType

NCORES = 8
B, L, D = 32, 300, 1024
G, C, DE, GD = 8, 64, 2048, 256
BL = 4                      # batches per core
VALID = [128, 128, 44]      # token tiles per batch
NKT = BL * 3                # 12 token tiles per core
NROWS_BN1 = B * L * G       # 76800
EPS_BN = 1e-5
QPC = 16                    # (h, c) channel-groups per core (16 * 128 chans)

_CACHE = {}


def build_kernel(has_be: bool, has_bias_cat: bool, n_cores: int = NCORES,
                 reps: int = 1):
    key = (has_be, has_bias_cat, n_cores, reps)
    if key in _CACHE:
        return _CACHE[key]

    nc = bacc.Bacc("TRN2", target_bir_lowering=False, debug=False,
                   num_devices=n_cores)

    xt_d = nc.dram_tensor("xt", [NKT, 128, D], BF16, kind="ExternalInput")
    xtt_d = nc.dram_tensor("xtt", [NKT, 128, 8, 128], BF16,
                           kind="ExternalInput")
    wet_d = nc.dram_tensor("wet", [8, 128, DE], BF16, kind="ExternalInput")
    wcat_d = nc.dram_tensor("wcat", [8, 128, 520], BF16, kind="ExternalInput")
    wlt_d = nc.dram_tensor("wlt", [QPC, 128, 1024], BF16, kind="ExternalInput")
    centt4_d = nc.dram_tensor("centt4", [2, 128, 256], F32,
                              kind="ExternalInput")
    fbnp_d = nc.dram_tensor("fbnp", [32, 128], F32, kind="ExternalInput")
    bn1gb_d = nc.dram_tensor("bn1gb", [1, 128], F32, kind="ExternalInput")
    bl8_d = nc.dram_tensor("bl8", [1, 1024], F32, kind="ExternalInput")
    if has_bias_cat:
        bias_cat_d = nc.dram_tensor("biascat", [1, 520], F32,
                                    kind="ExternalInput")
    if has_be:
        bet_d = nc.dram_tensor("bet", [2, 8, 128], F32, kind="ExternalInput")
    out_d = nc.dram_tensor("out", [32, 1024], F32, kind="ExternalOutput")

    group = [list(range(n_cores))]

    def _collective(kind, op, ins, outs):
        if n_cores == 1:
            nc.scalar.dma_start(out=outs[0], in_=ins[0])
        else:
            nc.gpsimd.collective_compute(kind, op, replica_groups=group,
                                         ins=[ins[0].opt()], outs=[outs[0].opt()])

    with tile.TileContext(nc) as tc:
      for _rep in range(reps):
            with tc.tile_pool(name="const", bufs=1) as cpool, \
                 tc.tile_pool(name="dram", bufs=1, space="DRAM") as dpool, \
                 tc.tile_pool(name="sa", bufs=NKT) as sapool, \
                 tc.tile_pool(name="xh", bufs=NKT) as xhpool, \
                 tc.tile_pool(name="gred", bufs=NKT) as gredpool, \
                 tc.tile_pool(name="wet", bufs=1) as wetpool:

                # ---------- P0: weights + constants ----------
                wcat_sb = cpool.tile([128, 8, 520], BF16)
                for wk in range(8):
                    nc.sync.dma_start(out=wcat_sb[:, wk, :],
                                      in_=wcat_d[wk, :, :])

                ident = cpool.tile([128, 128], F32)
                make_identity(nc, ident)
                ones_f = cpool.tile([128, 1], F32)
                nc.vector.memset(ones_f, 1.0)
                onesn_f = cpool.tile([128, 1], F32)
                nc.vector.memset(onesn_f, 1.0 / float(NROWS_BN1))
                onesn = cpool.tile([128, 1], F32R)
                nc.vector.tensor_copy(out=onesn, in_=onesn_f)
                ones_r = cpool.tile([128, 1], F32R)
                nc.vector.tensor_copy(out=ones_r, in_=ones_f)
                ones_bf = cpool.tile([128, 1], BF16)
                nc.vector.tensor_copy(out=ones_bf, in_=ones_f)
                epsbn = cpool.tile([128, 1], F32)
                nc.vector.memset(epsbn, EPS_BN)
                eps12 = cpool.tile([128, 1], F32)
                nc.vector.memset(eps12, 1e-12)

                centt4_sb = cpool.tile([128, 2, 256], F32)
                nc.scalar.dma_start(
                    out=centt4_sb,
                    in_=centt4_d[:, :, :].rearrange("j p c -> p j c"))
                bn1gb_sb = cpool.tile([1, 128], F32)
                nc.scalar.dma_start(out=bn1gb_sb, in_=bn1gb_d[:, :])
                bl8_row = cpool.tile([1, 1024], F32)
                nc.scalar.dma_start(out=bl8_row, in_=bl8_d[:, :])
                bl8_bc = cpool.tile([32, 1024], F32)
                nc.gpsimd.partition_broadcast(bl8_bc, bl8_row)
                fbnp_sb = cpool.tile([32, 128], F32)
                nc.scalar.dma_start(out=fbnp_sb, in_=fbnp_d[:, :])

                if has_bias_cat:
                    bc_row = cpool.tile([1, 520], F32)
                    nc.scalar.dma_start(out=bc_row, in_=bias_cat_d[:, :])
                    bc_bc = cpool.tile([128, 520], F32)
                    nc.gpsimd.partition_broadcast(bc_bc, bc_row)
                if has_be:
                    bet_sb = cpool.tile([128, 2, 8], F32)
                    nc.scalar.dma_start(
                        out=bet_sb,
                        in_=bet_d[:, :, :].rearrange("j g p -> p j g"))

                s_all = cpool.tile([128, NKT], F32)
                sig_all = cpool.tile([128, NKT, 8], F32)
                stats_sb = cpool.tile([1, 128], F32)
                gstats_sb = cpool.tile([1, 128], F32)
                scale_bc = cpool.tile([128, 512], F32)
                shift_bc = cpool.tile([128, 512], F32)
                y_all = cpool.tile([128, 8, 8, 64, 4], BF16)
                vladT = cpool.tile([128, 128, BL], F32)  # [d_low, (h c), b]

                wet_sb = wetpool.tile([128, 8, DE], BF16)
                wlt_sb = wetpool.tile([128, QPC, 1024], BF16)

                # DRAM bounce buffers
                stats_in = dpool.tile([1, 128], F32)
                stats_out = dpool.tile([1, 128], F32)
                a2a_in = dpool.tile([NCORES, 128, QPC, BL], F32)
                a2a_out = dpool.tile([NCORES, 128, QPC, BL], F32)
                ar_in = dpool.tile([32, 1024], F32)
                ar_out = dpool.tile([32, 1024], F32)

                # transposed final-BN params [128, 32] (cols 0:16 g, 16:32 b)
                with tc.tile_pool(name="p0ps", bufs=1, space="PSUM") as p0ps:
                    fps = p0ps.tile([128, 32], F32)
                    nc.tensor.transpose(fps, fbnp_sb, ident[:32, :32])
                    fbnT = cpool.tile([128, 32], F32)
                    nc.vector.tensor_copy(out=fbnT, in_=fps)

                sa_tiles = []
                xh_tiles = []
                gred_tiles = []

                # ---------- P1: sa matmuls + BN1 partial stats ----------
                with tc.tile_pool(name="p1xt", bufs=3) as p1xt, \
                     tc.tile_pool(name="p1xtt", bufs=3) as p1xtt, \
                     tc.tile_pool(name="p1scr", bufs=2) as p1scr, \
                     tc.tile_pool(name="p1sq", bufs=2) as p1sq, \
                     tc.tile_pool(name="p1small", bufs=4) as p1small, \
                     tc.tile_pool(name="p1ps", bufs=2, space="PSUM") as p1ps, \
                     tc.tile_pool(name="p1stats", bufs=1, space="PSUM") as p1statsps:

                    stats1 = p1statsps.tile([1, 512], F32, tag="st1")
                    stats2 = p1statsps.tile([1, 512], F32, tag="st2")

                    for kt in range(NKT):
                        ci = kt % 3
                        K = VALID[ci]

                        xt_t = p1xt.tile([128, D], BF16, tag="xt")
                        nc.sync.dma_start(out=xt_t, in_=xt_d[kt, :, :])
                        xT = p1xtt.tile([128, 8, 128], BF16, tag="xT")
                        nc.sync.dma_start(out=xT, in_=xtt_d[kt, :, :, :])

                        # token norms: s = rsqrt(sum x^2)
                        scr = p1scr.tile([128, D], BF16, tag="scr")
                        ssq = p1small.tile([128, 1], F32, tag="ssq")
                        nc.scalar.activation(out=scr, in_=xt_t, func=AF.Square,
                                             accum_out=ssq)
                        nrm = p1small.tile([128, 1], F32, tag="nrm")
                        nc.scalar.activation(out=nrm, in_=ssq, func=AF.Sqrt)
                        nc.vector.reciprocal(out=s_all[:, kt:kt + 1], in_=nrm)

                        # sa = xT.T @ wcat  (accumulate over d chunks)
                        saps = p1ps.tile([128, 520], F32, tag="saps")
                        for k in range(8):
                            nc.tensor.matmul(saps[:, 0:512], xT[:, k, :],
                                             wcat_sb[:, k, 0:512],
                                             start=(k == 0), stop=(k == 7))
                            nc.tensor.matmul(saps[:, 512:520], xT[:, k, :],
                                             wcat_sb[:, k, 512:520],
                                             start=(k == 0), stop=(k == 7))

                        sa_t = sapool.tile([128, 520], F32R, tag="sa")
                        sa_tiles.append(sa_t)
                        nc.vector.tensor_scalar_mul(
                            out=sa_t, in0=saps, scalar1=s_all[:, kt:kt + 1])
                        if has_bias_cat:
                            nc.vector.tensor_tensor(
                                out=sa_t.bitcast(F32), in0=sa_t.bitcast(F32),
                                in1=bc_bc, op=ALU.add)

                        # normalized tokens for stage1 (token-major, bf16)
                        xh = xhpool.tile([128, D], BF16, tag="xh")
                        xh_tiles.append(xh)
                        nc.vector.tensor_scalar_mul(
                            out=xh, in0=xt_t, scalar1=s_all[:, kt:kt + 1])

                        sq = p1sq.tile([128, 512], F32R, tag="sq")
                        nc.vector.tensor_mul(out=sq,
                                             in0=sa_t.bitcast(F32)[:, 0:512],
                                             in1=sa_t.bitcast(F32)[:, 0:512])
                        nc.tensor.matmul(stats1, onesn[:K], sa_t[:K, 0:512],
                                         start=(kt == 0), stop=(kt == NKT - 1))
                        nc.tensor.matmul(stats2, onesn[:K], sq[:K],
                                         start=(kt == 0), stop=(kt == NKT - 1))

                    # stream in the stage2 / final weights while softmax and
                    # stage1 run (DMA fabric is otherwise idle in P3)
                    for wk in range(8):
                        nc.sync.dma_start(out=wet_sb[:, wk, :],
                                          in_=wet_d[wk, :, :])
                    for q in range(QPC):
                        nc.sync.dma_start(out=wlt_sb[:, q, :],
                                          in_=wlt_d[q, :, :])

                    # chunk-reduce stats to 64 channels: channel j = col%64
                    nc.vector.tensor_reduce(
                        out=stats_sb[0:1, 0:64],
                        in_=stats1.rearrange("p (ch j) -> p j ch", ch=8),
                        axis=AX.X, op=ALU.add)
                    nc.vector.tensor_reduce(
                        out=stats_sb[0:1, 64:128],
                        in_=stats2.rearrange("p (ch j) -> p j ch", ch=8),
                        axis=AX.X, op=ALU.add)
                    nc.scalar.dma_start(out=stats_in, in_=stats_sb)
                    _collective("AllReduce", ALU.add, [stats_in], [stats_out])
                    nc.scalar.dma_start(out=gstats_sb, in_=stats_out)

                # ---------- BN1 global affine ----------
                # gstats = [mu | m2] (1/N folded into the stats stationary)
                mu = gstats_sb[0:1, 0:64]
                var = cpool.tile([1, 64], F32)
                nc.vector.tensor_mul(out=var, in0=mu, in1=mu)
                nc.vector.tensor_sub(out=var, in0=gstats_sb[0:1, 64:128],
                                     in1=var)
                sd = cpool.tile([1, 64], F32)
                nc.scalar.activation(out=sd, in_=var, func=AF.Sqrt,
                                     bias=epsbn[0:1])
                rstd = cpool.tile([1, 64], F32)
                nc.vector.reciprocal(out=rstd, in_=sd)
                scale_r = cpool.tile([1, 64], F32)
                nc.vector.tensor_mul(out=scale_r, in0=bn1gb_sb[0:1, 0:64],
                                     in1=rstd)
                shift_r = cpool.tile([1, 64], F32)
                nc.vector.tensor_mul(out=shift_r, in0=mu, in1=scale_r)
                nc.vector.tensor_sub(out=shift_r, in0=bn1gb_sb[0:1, 64:128],
                                     in1=shift_r)
                # tile 8x along the free dim, then broadcast to all partitions
                scale_cols = cpool.tile([1, 512], F32)
                nc.vector.tensor_copy(
                    out=scale_cols.rearrange("p (ch j) -> p ch j", ch=8),
                    in_=bass.AP(tensor=scale_r.tensor, offset=scale_r.offset,
                                ap=[scale_r.ap[0], [0, 8], [1, 64]]))
                shift_cols = cpool.tile([1, 512], F32)
                nc.vector.tensor_copy(
                    out=shift_cols.rearrange("p (ch j) -> p ch j", ch=8),
                    in_=bass.AP(tensor=shift_r.tensor, offset=shift_r.offset,
                                ap=[shift_r.ap[0], [0, 8], [1, 64]]))
                nc.gpsimd.partition_broadcast(scale_bc, scale_cols)
                nc.gpsimd.partition_broadcast(shift_bc, shift_cols)

                # attention gates: sigmoid = 1/(1+exp(-logit)); first Exp
                # triggers the exp table-set load (all rsqrts are done)
                with tc.tile_pool(name="p1c", bufs=4) as p1c:
                    for kt in range(NKT):
                        ea = p1c.tile([128, 8], F32, tag="ea")
                        nc.scalar.activation(
                            out=ea, in_=sa_tiles[kt].bitcast(F32)[:, 512:520],
                            func=AF.Exp, scale=-1.0)
                        nc.vector.tensor_scalar_add(out=ea, in0=ea,
                                                    scalar1=ones_f)
                        nc.vector.reciprocal(out=sig_all[:, kt, :], in_=ea)

                # ---------- P3: softmax + stage1 y + stage2 ft + vlad ------
                act_bufs = NKT if has_be else 3
                act_tiles = []
                with tc.tile_pool(name="p3e", bufs=2) as p3e, \
                     tc.tile_pool(name="p3act", bufs=act_bufs) as p3act, \
                     tc.tile_pool(name="p3small", bufs=6) as p3small, \
                     tc.tile_pool(name="p3vlad", bufs=4) as p3vlad:
                  with tc.tile_pool(name="p3ps", bufs=1, space="PSUM") as p3ps:

                    for b in range(BL):
                        ys = [None] * 8
                        for ci in range(3):
                            kt = b * 3 + ci
                            K = VALID[ci]
                            sa_t = sa_tiles[kt]

                            # z = sa*scale + shift ; e = exp(z)
                            e = p3e.tile([128, 512], F32, tag="e")
                            nc.gpsimd.tensor_tensor(
                                out=e, in0=sa_t.bitcast(F32)[:, 0:512],
                                in1=scale_bc, op=ALU.mult)
                            nc.vector.tensor_tensor(out=e, in0=e, in1=shift_bc,
                                                    op=ALU.add)
                            nc.scalar.activation(out=e, in_=e, func=AF.Exp)

                            den = p3small.tile([128, 8], F32, tag="den")
                            nc.vector.tensor_reduce(
                                out=den,
                                in_=e.rearrange("p (c g) -> p g c", g=8),
                                axis=AX.X, op=ALU.add)
                            rden = p3small.tile([128, 8], F32, tag="rden")
                            nc.vector.reciprocal(out=rden, in_=den)
                            w = p3small.tile([128, 8], F32, tag="w")
                            nc.vector.tensor_mul(out=w,
                                                 in0=sig_all[:, kt, :],
                                                 in1=rden)

                            # act = e * w (broadcast over c), bf16
                            act_t = p3act.tile([128, 512], BF16, tag="act")
                            act_tiles.append(act_t)
                            nc.vector.tensor_tensor(
                                out=act_t.rearrange("p (c g) -> p c g", g=8),
                                in0=e.rearrange("p (c g) -> p c g", g=8),
                                in1=bass.AP(tensor=w.tensor, offset=w.offset,
                                            ap=[w.ap[0], [0, 64], [1, 8]]),
                                op=ALU.mult)
                            # per-token sum over g (for a_sum)
                            gred = gredpool.tile([128, 64], BF16, tag="gred")
                            gred_tiles.append(gred)
                            with nc.allow_low_precision(
                                    reason="8-term reduce rounded to bf16"):
                                nc.vector.tensor_reduce(
                                    out=gred,
                                    in_=act_t.rearrange("p (c g) -> p c g",
                                                        g=8),
                                    axis=AX.X, op=ALU.add)

                            # stage1: y[d_chunk, (c g)] += xh_chunk.T @ act
                            for k in range(8):
                                if ci == 0:
                                    ys[k] = p3ps.tile([128, 512], F32,
                                                      name=f"y{k}_{b}",
                                                      tag=f"y{k}")
                                nc.tensor.matmul(
                                    ys[k], xh_tiles[kt][:K, k * 128:(k + 1) * 128],
                                    act_t[:K, :], start=(ci == 0),
                                    stop=(ci == 2))

                        # copy y out of PSUM into the bf16 staging buffer:
                        # y_all[d_low, k, g, c, b]
                        for k in range(8):
                            dst = bass.AP(
                                tensor=y_all.tensor,
                                offset=y_all.offset + (k * 2048 + b),
                                ap=[y_all.ap[0], [4, 64], [256, 8]])
                            src = ys[k].rearrange("p (c g) -> p c g", g=8)
                            if k % 2 == 0:
                                nc.vector.tensor_copy(out=dst, in_=src)
                            else:
                                nc.scalar.copy(out=dst, in_=src)

                  # ---------- stage2: ft[d', (c b)] = sum_g We_g^T @ y_g --
                  with tc.tile_pool(name="p3ps2", bufs=1,
                                    space="PSUM") as p3ps2:
                    ft = [p3ps2.tile([128, 256], F32, name=f"ft{j}",
                                     tag=f"ft{j}")
                          for j in range(2)]
                    asum_ps = p3ps2.tile([1, 256], F32, tag="asum")
                    ssq_ps = p3ps2.tile([1, 256], F32, tag="ssq")

                    # a_sum rows first (PE): asum[0, 4c+b] = sum_l gred[l, c],
                    # so the broadcast chain below overlaps the stage2 matmuls
                    for kt in range(NKT):
                        bb, ci = divmod(kt, 3)
                        K = VALID[ci]
                        dst = bass.AP(tensor=asum_ps.tensor,
                                      offset=asum_ps.offset + bb,
                                      ap=[asum_ps.ap[0], [4, 64]])
                        nc.tensor.matmul(dst, ones_bf[:K],
                                         gred_tiles[kt][:K, :],
                                         start=(ci == 0), stop=(ci == 2))

                    for k in range(8):
                        for g in range(8):
                            for j in range(2):
                                nc.tensor.matmul(
                                    ft[j], wet_sb[:, k, g * 256 + j * 128:
                                                  g * 256 + (j + 1) * 128],
                                    y_all[:, k, g, :, :],
                                    start=(k == 0 and g == 0),
                                    stop=(k == 7 and g == 7))

                    asum_row = cpool.tile([1, 256], F32)
                    nc.vector.tensor_copy(out=asum_row, in_=asum_ps)
                    asum_bc = cpool.tile([128, 256], F32)
                    nc.gpsimd.partition_broadcast(asum_bc, asum_row)

                    if has_be:
                        # asg[b][c*8+g] = sum_l act[l, (c g)]
                        asg_ps = p3ps2.tile([4, 512], F32, tag="asg")
                        for kt in range(NKT):
                            bb, ci = divmod(kt, 3)
                            K = VALID[ci]
                            nc.tensor.matmul(asg_ps[bb:bb + 1, :],
                                             ones_bf[:K],
                                             act_tiles[kt][:K, :],
                                             start=(ci == 0), stop=(ci == 2))
                        asg_sb = cpool.tile([4, 512], F32)
                        nc.vector.tensor_copy(out=asg_sb, in_=asg_ps)
                        asg_bcs = []
                        for bb in range(BL):
                            abc = cpool.tile([128, 512], F32)
                            nc.gpsimd.partition_broadcast(
                                abc, asg_sb[bb:bb + 1, :])
                            asg_bcs.append(abc)

                    vladU = [None, None]
                    for j in range(2):
                        sterm = p3vlad.tile([128, 256], F32, tag="sterm")
                        nc.vector.tensor_mul(out=sterm, in0=asum_bc,
                                             in1=centt4_sb[:, j, :])
                        vU = p3vlad.tile([128, 256], F32, tag="vladU")
                        vladU[j] = vU
                        nc.vector.tensor_sub(out=vU, in0=ft[j], in1=sterm)
                        if has_be:
                            # vlad[d',(c b)] += sum_g asg[c,g]*be[g*256+j*128+d']
                            tmp = p3vlad.tile([128, 64], F32, tag="betmp")
                            for bb in range(BL):
                                vslice = bass.AP(
                                    tensor=vU.tensor, offset=vU.offset + bb,
                                    ap=[vU.ap[0], [4, 64]])
                                for g in range(G):
                                    aslice = bass.AP(
                                        tensor=asg_bcs[bb].tensor,
                                        offset=asg_bcs[bb].offset + g,
                                        ap=[asg_bcs[bb].ap[0], [8, 64]])
                                    nc.vector.tensor_scalar_mul(
                                        out=tmp, in0=aslice,
                                        scalar1=bet_sb[:, j, g:g + 1])
                                    nc.vector.tensor_tensor(
                                        out=vslice, in0=vslice, in1=tmp,
                                        op=ALU.add)
                        vsq = p3vlad.tile([128, 256], F32R, tag="vsq")
                        nc.vector.tensor_mul(out=vsq, in0=vU, in1=vU)
                        nc.tensor.matmul(ssq_ps, ones_r, vsq,
                                         start=(j == 0), stop=(j == 1))

                    # rn = 1/max(sqrt(ssq), 1e-12)
                    nrm2 = cpool.tile([1, 256], F32)
                    nc.scalar.activation(out=nrm2, in_=ssq_ps, func=AF.Sqrt)
                    nc.vector.tensor_scalar_max(out=nrm2, in0=nrm2,
                                                scalar1=eps12[0:1])
                    rn_row = cpool.tile([1, 256], F32)
                    nc.vector.reciprocal(out=rn_row, in_=nrm2)
                    rn_bc = cpool.tile([128, 256], F32)
                    nc.gpsimd.partition_broadcast(rn_bc, rn_row)
                    for j in range(2):
                        nc.vector.tensor_tensor(
                            out=vladT[:, j * 64:(j + 1) * 64, :],
                            in0=vladU[j].rearrange("p (c b) -> p c b", b=4),
                            in1=rn_bc.rearrange("p (c b) -> p c b", b=4),
                            op=ALU.mult)

                    nc.sync.dma_start(
                        out=a2a_in[:, :, :, :].rearrange("d p q b -> p d q b"),
                        in_=vladT.rearrange("p (d q) b -> p d q b", d=NCORES))
                    _collective("AllToAll", ALU.bypass, [a2a_in], [a2a_out])

                # ---------- P4: final BN + final matmul ----------
                with tc.tile_pool(name="p4", bufs=2) as p4pool, \
                     tc.tile_pool(name="p4small", bufs=8) as p4small, \
                     tc.tile_pool(name="p4ps", bufs=2, space="PSUM") as p4ps:

                    vchunk = p4pool.tile([128, NCORES, QPC, BL], F32,
                                         tag="vchunk")
                    nc.sync.dma_start(
                        out=vchunk,
                        in_=a2a_out[:, :, :, :].rearrange("s p q b -> p s q b"))
                    vbn = p4pool.tile([128, QPC, 32], BF16, tag="vbn")

                    for q in range(QPC):
                        vflat = p4small.tile([128, 32], F32, tag="vflat")
                        nc.vector.tensor_copy(
                            out=vflat.rearrange("p (s b) -> p s b", b=BL),
                            in_=vchunk[:, :, q, :])
                        bnst = p4small.tile([128, 6], F32, tag="bnst")
                        nc.vector.bn_stats(out=bnst, in_=vflat)
                        mv = p4small.tile([128, 2], F32, tag="mv")
                        nc.vector.bn_aggr(out=mv, in_=bnst)
                        sdq = p4small.tile([128, 1], F32, tag="sdq")
                        nc.scalar.activation(out=sdq, in_=mv[:, 1:2],
                                             func=AF.Sqrt, bias=epsbn)
                        rsq = p4small.tile([128, 1], F32, tag="rsq")
                        nc.vector.reciprocal(out=rsq, in_=sdq)
                        scq = p4small.tile([128, 1], F32, tag="scq")
                        nc.vector.tensor_mul(out=scq, in0=fbnT[:, q:q + 1],
                                             in1=rsq)
                        shq = p4small.tile([128, 1], F32, tag="shq")
                        nc.vector.tensor_mul(out=shq, in0=mv[:, 0:1], in1=scq)
                        nc.vector.tensor_sub(out=shq,
                                             in0=fbnT[:, 16 + q:17 + q],
                                             in1=shq)
                        nc.vector.tensor_scalar(out=vbn[:, q, :], in0=vflat,
                                                scalar1=scq, scalar2=shq,
                                                op0=ALU.mult, op1=ALU.add)

                    out_sb = p4pool.tile([32, 1024], F32, tag="outsb")
                    for n in range(2):
                        fpsm = p4ps.tile([32, 512], F32, tag="fin")
                        for q in range(QPC):
                            nc.tensor.matmul(
                                fpsm, vbn[:, q, :],
                                wlt_sb[:, q, n * 512:(n + 1) * 512],
                                start=(q == 0), stop=(q == QPC - 1))
                        nc.vector.tensor_tensor(
                            out=out_sb[:, n * 512:(n + 1) * 512], in0=fpsm,
                            in1=bl8_bc[:, n * 512:(n + 1) * 512], op=ALU.add)

                    nc.scalar.dma_start(out=ar_in, in_=out_sb)
                    _collective("AllReduce", ALU.add, [ar_in], [ar_out])
                    nc.scalar.dma_start(out=out_d[:, :], in_=ar_out)

    nc.finalize()
    _CACHE[key] = nc
    return nc


def _prep_inputs(x, We, be, Ws, bn1_g, bn1_b, Wa, ba, centroids,
                 fbn_g, fbn_b, Wl, bl):
    f = np.float32
    x = np.asarray(x, f)
    We = np.asarray(We, f)
    Ws = np.asarray(Ws, f)
    Wa = np.asarray(Wa, f)
    be = np.asarray(be, f)
    ba = np.asarray(ba, f)
    Wl = np.asarray(Wl, f)

    WsWe = Ws @ We                       # (512, 1024)
    WaWe = Wa @ We                       # (8, 1024)
    Wcat = np.concatenate([WsWe, WaWe], 0)          # (520, 1024)
    WcatT = np.ascontiguousarray(Wcat.T).reshape(8, 128, 520).astype(bfloat16)
    WeT = np.ascontiguousarray(We.T).reshape(8, 128, DE).astype(bfloat16)

    bias_cat = np.concatenate([Ws @ be, Wa @ be + ba]).reshape(1, 520)
    has_bias_cat = bool(np.any(bias_cat))
    has_be = bool(np.any(be))

    # permuted channel order: p_idx = (h*64 + c)*128 + d_low,
    # original chan = c*256 + h*128 + d_low
    Wlp = np.ascontiguousarray(
        Wl.reshape(1024, C, 2, 128).transpose(2, 1, 3, 0).reshape(16384, 1024))
    fg = np.ascontiguousarray(
        np.asarray(fbn_g, f).reshape(C, 2, 128).transpose(1, 0, 2).reshape(128, 128))
    fb = np.ascontiguousarray(
        np.asarray(fbn_b, f).reshape(C, 2, 128).transpose(1, 0, 2).reshape(128, 128))

    bn1gb = np.concatenate([np.asarray(bn1_g, f),
                            np.asarray(bn1_b, f)]).reshape(1, 128)
    bl8 = (np.asarray(bl, f) / 8.0).reshape(1, 1024)
    # centt4[j, d_low, 4c+b] = centroids[c, j*128 + d_low]
    cent = np.asarray(centroids, f)                  # (C, GD)
    centt4 = np.ascontiguousarray(
        np.repeat(cent.reshape(C, 2, 128).transpose(1, 2, 0), BL,
                  axis=2).reshape(2, 128, C, BL).reshape(2, 128, 256))
    bet = np.ascontiguousarray(be.reshape(G, 2, 128).transpose(1, 0, 2))

    in_maps = []
    for j in range(NCORES):
        xj = x[j * BL:(j + 1) * BL]          # (4, 300, 1024)
        xt = np.ones((NKT, 128, D), f)
        for b in range(BL):
            for ci in range(3):
                v = VALID[ci]
                xt[b * 3 + ci, :v] = xj[b, ci * 128:ci * 128 + v]
        xtb = xt.astype(bfloat16)
        xtt = np.ascontiguousarray(
            xtb.reshape(NKT, 128, 8, 128).transpose(0, 3, 2, 1))
        fbnp = np.concatenate([fg[j * QPC:(j + 1) * QPC],
                               fb[j * QPC:(j + 1) * QPC]], 0)  # (32, 128)
        wlt = np.ascontiguousarray(
            Wlp[j * 2048:(j + 1) * 2048].reshape(QPC, 128, 1024)).astype(bfloat16)
        m = {"xt": np.ascontiguousarray(xtb), "xtt": xtt,
             "wet": WeT, "wcat": WcatT,
             "wlt": wlt, "centt4": centt4, "fbnp": np.ascontiguousarray(fbnp),
             "bn1gb": bn1gb, "bl8": bl8}
        if has_bias_cat:
            m["biascat"] = bias_cat
        if has_be:
            m["bet"] = bet
        in_maps.append(m)
    return in_maps, has_be, has_bias_cat


def kernel(**inputs):
    in_maps, has_be, has_bias_cat = _prep_inputs(**inputs)
    nc = build_kernel(has_be, has_bias_cat)
    res = run_bass_kernel_spmd(nc, in_maps, core_ids=list(range(NCORES)))
    out = np.ascontiguousarray(np.asarray(res.results[0]["out"], np.float32))
    return out
